# revision 1
# baseline (speedup 1.0000x reference)
"""Trainium2 Bass kernel for CommutatorConv2d.

Math: with lambda_c=0, lambda_a=1 the reference is a conv2d with effective
kernel  w_eff[o,i,r,s] = krow[o,i,s] + kcol[o,i,r]  (krow = sum_r w, kcol =
sum_s w), plus bias.  That kernel lives in a 6-dim matrix subspace, so the
9-tap conv factors into two 1D convs over box-summed inputs:

  y[o,h,w] = sum_{i,s} krow[o,i,s] * xv[i, h, w+s-1]
           + sum_{i,r} kcol[o,i,r] * xh[i, h+r-1, w]  + bias[o]

where xv = vertical 3-tap sum of zero-padded x, xh = horizontal 3-tap sum.
Per output tile that is 6 accumulating matmuls (contraction 128 each)
instead of 9 — 2/3 of the PE work of direct conv.

Sharding: data-parallel over batch; 4 images per core on 8 cores.

Schedule notes (from neuron-profile traces):
- All DMAs issue from the sync queue in priority order (head chunk,
  half-0 weights, half-1 weights, bias, remaining chunks): the DMA
  engines drain one queue's descriptors in order, so the transfers that
  gate the first matmul complete first.  Splitting issues across two
  queues makes everything round-robin and finish late.
- Dummy matmuls bridge the tensor engine from the framework preamble to
  the first real tile with no idle gap; any gap decays the HAM p-state /
  utilization limit and costs a ~6.8us half-rate re-ramp over real work.
- Box-sums are emitted per row-range so each arriving DMA chunk unlocks
  tiles immediately; image 0 interleaves output-channel halves.
- Output is stored as bf16 (host upcasts): halves store traffic, and the
  final half-image ends in two 4-row tiles so the kernel tail only waits
  on a 224-column activation + store.
"""

import os
import numpy as np
import ml_dtypes

import concourse.bass as bass
import concourse.bacc as bacc
import concourse.mybir as mybir
import concourse.tile as tile
from concourse.bass_utils import run_bass_kernel_spmd

B, CI, CO, H, W = 32, 128, 256, 56, 56
NCORES = 8
BPC = B // NCORES          # images per core
HP, WP = H + 2, W + 2      # padded spatial dims
NPIX = H * W               # 3136
ROWT = 8                   # output rows per matmul tile
NT = H // ROWT             # 7 pixel tiles per image
NTILE = ROWT * W           # 448 columns per matmul

ROW_CHUNKS0 = [10, 26, HP]  # image-0 row chunks; chunk to row r unlocks tiles t with 8t+10 <= r
# box-sum sub-splits per chunk: compute xv/xh for the first unlocked tile's
# rows first so the PE never waits on a full-chunk vector op
SUM_SPLITS = {26: [18, 26], HP: [42, HP]}
N_WARM = 33                 # PE warmup matmuls (bridge idle->real work, keeps HAM limit up)
WARMC = 128                 # columns per warmup matmul

F32 = mybir.dt.float32
BF16 = mybir.dt.bfloat16


def build_nc():
    nc = bacc.Bacc(None, enable_partition_id=False)
    xin = nc.declare_dram_parameter("xp", [BPC, CI, HP, WP], BF16, isOutput=False)
    wk = nc.declare_dram_parameter("klhs", [CI, 2, 6, 128], BF16, isOutput=False)
    bb = nc.declare_dram_parameter("bias2", [CI, 2], F32, isOutput=False)
    y = nc.declare_dram_parameter("y", [BPC, CO, H, W], BF16, isOutput=True)

    xflat = xin.rearrange("b c h w -> b c (h w)")
    yflat = y.rearrange("b o h w -> b o (h w)")
    wkflat = wk.rearrange("i h t o -> i (h t o)")
    NPAD = HP * WP           # 3364
    NV = H * WP              # 3248 (rows 0..55 of padded, all 58 cols)

    with tile.TileContext(nc) as tc:
        with (
            tc.tile_pool(name="const", bufs=1) as cpool,
            tc.tile_pool(name="xp", bufs=2) as xpool,
            tc.tile_pool(name="xv", bufs=2) as vpool,
            tc.tile_pool(name="xh", bufs=2) as hpool,
            tc.tile_pool(name="yo", bufs=4) as ypool,
            tc.tile_pool(name="ps", bufs=7, space="PSUM") as pspool,
        ):
            klhs_sb = cpool.tile([CI, 2 * 6 * 128], BF16)
            bias_sb = cpool.tile([CI, 2], F32)
            kl4 = klhs_sb.rearrange("i (h t o) -> i h t o", t=6, o=128)

            # PE warmup: dummy matmuls issued while the first input DMAs are
            # in flight keep the tensor engine active so the HAM utilization
            # limit ramp overlaps the DMA wait instead of the real matmuls.
            warm = cpool.tile([128, WARMC], BF16)
            nc.gpsimd.memset(warm[:], 0.0)
            warm_ps = pspool.tile([128, WARMC], F32, bufs=1, tag="warm")
            for _ in range(N_WARM):
                nc.tensor.matmul(
                    warm_ps[:], warm[:, 0:128], warm[:], start=True, stop=True
                )

            for b in range(BPC):
                row_chunks = ROW_CHUNKS0 if b == 0 else [HP]

                xp_sb = xpool.tile([CI, NPAD], BF16)
                xp3d = xflat[b].rearrange("i (h w) -> i h w", w=WP)
                xps3 = xp_sb.rearrange("i (h w) -> i h w", w=WP)
                r0 = 0
                for ci, r1 in enumerate(row_chunks):
                    # single queue, priority order: the DMA engines drain one
                    # queue's descriptors in order, so critical transfers
                    # (head chunk, first-half weights) complete first instead
                    # of round-robining with the bulk loads
                    nc.sync.dma_start(out=xps3[:, r0:r1, :], in_=xp3d[:, r0:r1, :])
                    if b == 0 and ci == 0:
                        nc.sync.dma_start(
                            out=klhs_sb[:, 0:768], in_=wkflat[:, 0:768]
                        )
                        nc.sync.dma_start(
                            out=klhs_sb[:, 768:1536], in_=wkflat[:, 768:1536]
                        )
                        nc.sync.dma_start(out=bias_sb[:], in_=bb[:])
                    r0 = r1

                # box-sums, emitted per DMA chunk so they overlap the loads:
                # xv[j] = xp[j] + xp[j+58] + xp[j+116]   (rows 0..55)
                # xh[j] = xp[j] + xp[j+1] + xp[j+2]      (rows 0..57, garbage
                #                                         at cols 56/57 unused)
                xvt = vpool.tile([CI, NV], BF16)
                xv = vpool.tile([CI, NV], BF16)
                xht = hpool.tile([CI, NPAD], BF16)
                xh = hpool.tile([CI, NPAD], BF16)
                bounds = []
                for r1 in row_chunks:
                    bounds.extend(SUM_SPLITS.get(r1, [r1]) if b == 0 else [r1])
                v0 = h0r = 0
                for s1 in bounds:
                    v1 = H if s1 == HP else s1 - 2    # xv rows ready
                    h1 = s1                           # xh rows ready
                    a, z = v0 * WP, v1 * WP
                    nc.vector.tensor_add(
                        xvt[:, a:z], xp_sb[:, a:z], xp_sb[:, a + WP : z + WP]
                    )
                    nc.vector.tensor_add(
                        xv[:, a:z], xvt[:, a:z], xp_sb[:, a + 2 * WP : z + 2 * WP]
                    )
                    a, z = h0r * WP, h1 * WP - 2
                    nc.vector.tensor_add(
                        xht[:, a:z], xp_sb[:, a:z], xp_sb[:, a + 1 : z + 1]
                    )
                    nc.vector.tensor_add(
                        xh[:, a:z], xht[:, a:z], xp_sb[:, a + 2 : z + 2]
                    )
                    v0, h0r = v1, h1

                xv3 = xv.rearrange("i (h w) -> i h w", w=WP)   # [128, 56, 58]
                xh3 = xh.rearrange("i (h w) -> i h w", w=WP)   # [128, 58, 58]

                youts = {}

                def emit(half, t, b=b, xv3=xv3, xh3=xh3, youts=youts):
                    if half not in youts:
                        youts[half] = ypool.tile(
                            [128, NPIX], BF16, name=f"yout_{b}_{half}", tag="yout"
                        )
                    yout = youts[half]
                    h0 = t * ROWT
                    ps = pspool.tile([128, NTILE], F32, name=f"ps_{b}_{half}_{t}", tag="ps")
                    for s in range(3):
                        nc.tensor.matmul(
                            ps[:],
                            kl4[:, half, s, :],
                            xv3[:, h0 : h0 + ROWT, s : s + W],
                            start=(s == 0),
                            stop=False,
                        )
                    for r in range(3):
                        nc.tensor.matmul(
                            ps[:],
                            kl4[:, half, 3 + r, :],
                            xh3[:, h0 + r : h0 + r + ROWT, 0:W],
                            start=False,
                            stop=(r == 2),
                        )
                    last_block = b == BPC - 1 and half == 1
                    if last_block and t == NT - 1:
                        # final tile: split activation + store into halves so
                        # the second store's issue overlaps the first's and
                        # the kernel tail only waits on a 224-column DMA
                        c0 = t * NTILE
                        for p0, p1 in ((0, NTILE // 2), (NTILE // 2, NTILE)):
                            nc.scalar.activation(
                                yout[:, c0 + p0 : c0 + p1],
                                ps[:, p0:p1],
                                mybir.ActivationFunctionType.Identity,
                                bias=bias_sb[:, half : half + 1],
                            )
                            nc.sync.dma_start(
                                out=yflat[
                                    b, half * 128 : half * 128 + 128, c0 + p0 : c0 + p1
                                ],
                                in_=yout[:, c0 + p0 : c0 + p1],
                            )
                        return
                    nc.scalar.activation(
                        yout[:, t * NTILE : (t + 1) * NTILE],
                        ps[:],
                        mybir.ActivationFunctionType.Identity,
                        bias=bias_sb[:, half : half + 1],
                    )
                    if t == 3:
                        nc.sync.dma_start(
                            out=yflat[b, half * 128 : half * 128 + 128, 0 : 4 * NTILE],
                            in_=yout[:, 0 : 4 * NTILE],
                        )
                    elif t >= 4 and last_block:
                        # final block: per-tile stores so the kernel tail
                        # only waits on small DMAs
                        nc.sync.dma_start(
                            out=yflat[
                                b,
                                half * 128 : half * 128 + 128,
                                t * NTILE : (t + 1) * NTILE,
                            ],
                            in_=yout[:, t * NTILE : (t + 1) * NTILE],
                        )
                    if t == NT - 1 and not last_block:
                        nc.sync.dma_start(
                            out=yflat[b, half * 128 : half * 128 + 128, 4 * NTILE : NPIX],
                            in_=yout[:, 4 * NTILE : NPIX],
                        )

                if b == 0:
                    # image 0: interleave halves so each arriving row chunk
                    # immediately unlocks two tiles of PE work
                    order = [(h, t) for t in range(NT) for h in range(2)]
                elif b == BPC - 1:
                    # last image: stop half 1 before its final 8-row tile;
                    # rows 48-55 are emitted below as two 4-row tiles so the
                    # kernel tail waits on a 224-column activation + store
                    order = [(0, t) for t in range(NT)] + [(1, t) for t in range(NT - 1)]
                else:
                    order = [(h, t) for h in range(2) for t in range(NT)]
                for half, t in order:
                    emit(half, t)
                if b == BPC - 1:
                    yout1 = youts[1]
                    for k in range(2):
                        h0 = (NT - 1) * ROWT + 4 * k
                        c0 = h0 * W
                        ps = pspool.tile([128, 4 * W], F32, name=f"ps_tail_{k}", tag="ps")
                        for s in range(3):
                            nc.tensor.matmul(
                                ps[:], kl4[:, 1, s, :],
                                xv3[:, h0 : h0 + 4, s : s + W],
                                start=(s == 0), stop=False,
                            )
                        for r in range(3):
                            nc.tensor.matmul(
                                ps[:], kl4[:, 1, 3 + r, :],
                                xh3[:, h0 + r : h0 + r + 4, 0:W],
                                start=False, stop=(r == 2),
                            )
                        nc.scalar.activation(
                            yout1[:, c0 : c0 + 4 * W], ps[:],
                            mybir.ActivationFunctionType.Identity,
                            bias=bias_sb[:, 1:2],
                        )
                        nc.sync.dma_start(
                            out=yflat[b, 128:256, c0 : c0 + 4 * W],
                            in_=yout1[:, c0 : c0 + 4 * W],
                        )

            # read the warm PSUM bank at the very end so the warmup matmuls
            # are never dead-code-eliminated but gate nothing
            warm_out = cpool.tile([128, 32], F32)
            nc.scalar.activation(
                warm_out[:], warm_ps[:, 0:32], mybir.ActivationFunctionType.Copy
            )
    nc.finalize()
    return nc


_NC_CACHE = {}


def _get_nc():
    if "nc" not in _NC_CACHE:
        _NC_CACHE["nc"] = build_nc()
    return _NC_CACHE["nc"]


def make_in_maps(x, weight, bias):
    x = np.asarray(x, dtype=np.float32)
    weight = np.asarray(weight, dtype=np.float32)
    bias = np.asarray(bias, dtype=np.float32)

    krow = weight.sum(axis=3)  # [O, I, 3]
    kcol = weight.sum(axis=2)  # [O, I, 3]
    klhs = np.empty((CI, 2, 6, 128), np.float32)
    for half in range(2):
        o0 = half * 128
        for s in range(3):
            klhs[:, half, s, :] = krow[o0 : o0 + 128, :, s].T
            klhs[:, half, 3 + s, :] = kcol[o0 : o0 + 128, :, s].T
    klhs = klhs.astype(ml_dtypes.bfloat16)

    xp = np.zeros((B, CI, HP, WP), np.float32)
    xp[:, :, 1 : H + 1, 1 : W + 1] = x
    xp = xp.astype(ml_dtypes.bfloat16)

    bias2 = np.ascontiguousarray(bias.reshape(2, 128).T)  # [128, 2] f32

    return [
        {"xp": xp[c * BPC : (c + 1) * BPC], "klhs": klhs, "bias2": bias2}
        for c in range(NCORES)
    ]


def run(in_maps, **kwargs):
    nc = _get_nc()
    return run_bass_kernel_spmd(nc, in_maps, list(range(NCORES)), **kwargs)


def kernel(x, weight, bias):
    res = run(make_in_maps(x, weight, bias))
    return np.concatenate(
        [res.results[c]["y"].astype(np.float32) for c in range(NCORES)], axis=0
    )



# revision 2
# speedup vs baseline: 1.0686x; 1.0686x over previous
"""Trainium2 Bass kernel for CommutatorConv2d.

Math: with lambda_c=0, lambda_a=1 the reference is a conv2d with effective
kernel  w_eff[o,i,r,s] = krow[o,i,s] + kcol[o,i,r]  (krow = sum_r w, kcol =
sum_s w), plus bias.  That kernel lives in a 5-dim matrix subspace
(row-functions + col-functions share the constants), so the 9-tap conv
factors into FIVE contraction-128 matmuls per output tile:

  y[o,h,w] = W1[o,i] @ xbox[i,h,w]            (xbox = 3x3 box sum of x)
           + d0[o,i] @ xv[i,h,w-1] + d2[o,i] @ xv[i,h,w+1]
           + e0[o,i] @ xh[i,h-1,w] + e2[o,i] @ xh[i,h+1,w]  + bias[o]

  where xv/xh are vertical/horizontal 3-tap sums of zero-padded x,
  d0 = krow0-krow1, d2 = krow2-krow1, e0 = kcol0-kcol1, e2 = kcol2-kcol1,
  W1 = krow1+kcol1  (the center taps absorbed into the box-sum term).
  All spatial shifts are free access-pattern reads; only xv, xh, xbox
  need vector ops (6 adds per image).  5 matmuls/tile instead of the
  9 of direct conv or 6 of the two-1D-conv factorization.

Sharding: data-parallel over batch; 4 images per core on 8 cores.

Schedule notes (from neuron-profile traces):
- All DMAs issue from the sync queue in priority order (head chunk,
  half-0 weights, half-1 weights, bias, remaining chunks): the DMA
  engines drain one queue's descriptors in order, so the transfers that
  gate the first matmul complete first.
- Dummy matmuls bridge the tensor engine from the framework preamble to
  the first real tile with no idle gap; any gap decays the HAM p-state /
  utilization limit and costs a re-ramp over real work.
- Box-sums are emitted per row-range so each arriving DMA chunk unlocks
  tiles immediately; image 0 interleaves output-channel halves.  Matmul
  order within a tile follows data readiness: xv taps, xh taps, xbox.
- Output is stored as bf16 (host upcasts): halves store traffic, and the
  final half-image ends in two 4-row tiles so the kernel tail only waits
  on a 224-column activation + store.
"""

import os
import numpy as np
import ml_dtypes

import concourse.bass as bass
import concourse.bacc as bacc
import concourse.mybir as mybir
import concourse.tile as tile
from concourse.bass_utils import run_bass_kernel_spmd

B, CI, CO, H, W = 32, 128, 256, 56, 56
NCORES = 8
BPC = B // NCORES          # images per core
HP, WP = H + 2, W + 2      # padded spatial dims
NPIX = H * W               # 3136
ROWT = 8                   # output rows per matmul tile
NT = H // ROWT             # 7 pixel tiles per image
NTILE = ROWT * W           # 448 columns per matmul
NTAP = 5                   # matmuls per tile

ROW_CHUNKS0 = [10, 26, HP]  # image-0 row chunks; chunk to row r unlocks tiles t with 8t+10 <= r
# box-sum sub-splits per chunk: compute sums for the first unlocked tile's
# rows first so the PE never waits on a full-chunk vector op
SUM_SPLITS = {26: [18, 26], HP: [42, HP]}
N_WARM = 33                 # PE warmup matmuls (bridge idle->real work, keeps HAM limit up)
WARMC = 128                 # columns per warmup matmul

F32 = mybir.dt.float32
BF16 = mybir.dt.bfloat16


def build_nc():
    nc = bacc.Bacc(None, enable_partition_id=False)
    xin = nc.declare_dram_parameter("xp", [BPC, CI, HP, WP], BF16, isOutput=False)
    wk = nc.declare_dram_parameter("klhs", [CI, 2, NTAP, 128], BF16, isOutput=False)
    bb = nc.declare_dram_parameter("bias2", [CI, 2], F32, isOutput=False)
    y = nc.declare_dram_parameter("y", [BPC, CO, H, W], BF16, isOutput=True)

    xflat = xin.rearrange("b c h w -> b c (h w)")
    yflat = y.rearrange("b o h w -> b o (h w)")
    wkflat = wk.rearrange("i h t o -> i (h t o)")
    NPAD = HP * WP           # 3364
    NV = H * WP              # 3248 (rows 0..55 of padded, all 58 cols)
    NW = NTAP * 128          # weight columns per half

    with tile.TileContext(nc) as tc:
        with (
            tc.tile_pool(name="const", bufs=1) as cpool,
            tc.tile_pool(name="xp", bufs=2) as xpool,
            tc.tile_pool(name="xv", bufs=2) as vpool,
            tc.tile_pool(name="xh", bufs=2) as hpool,
            tc.tile_pool(name="xb", bufs=2) as bpool,
            tc.tile_pool(name="yo", bufs=4) as ypool,
            tc.tile_pool(name="ps", bufs=7, space="PSUM") as pspool,
        ):
            klhs_sb = cpool.tile([CI, 2 * NW], BF16)
            bias_sb = cpool.tile([CI, 2], F32)
            kl4 = klhs_sb.rearrange("i (h t o) -> i h t o", t=NTAP, o=128)

            # PE warmup: dummy matmuls issued while the first input DMAs are
            # in flight keep the tensor engine active so the HAM utilization
            # limit ramp overlaps the DMA wait instead of the real matmuls.
            warm = cpool.tile([128, WARMC], BF16)
            nc.gpsimd.memset(warm[:], 0.0)
            warm_ps = pspool.tile([128, WARMC], F32, bufs=1, tag="warm")
            for _ in range(N_WARM):
                nc.tensor.matmul(
                    warm_ps[:], warm[:, 0:128], warm[:], start=True, stop=True
                )

            for b in range(BPC):
                row_chunks = ROW_CHUNKS0 if b == 0 else [HP]

                xp_sb = xpool.tile([CI, NPAD], BF16)
                xp3d = xflat[b].rearrange("i (h w) -> i h w", w=WP)
                xps3 = xp_sb.rearrange("i (h w) -> i h w", w=WP)
                r0 = 0
                for ci, r1 in enumerate(row_chunks):
                    # single queue, priority order: the DMA engines drain one
                    # queue's descriptors in order, so critical transfers
                    # (head chunk, first-half weights) complete first instead
                    # of round-robining with the bulk loads
                    nc.sync.dma_start(out=xps3[:, r0:r1, :], in_=xp3d[:, r0:r1, :])
                    if b == 0 and ci == 0:
                        nc.sync.dma_start(
                            out=klhs_sb[:, 0:NW], in_=wkflat[:, 0:NW]
                        )
                        nc.sync.dma_start(
                            out=klhs_sb[:, NW : 2 * NW], in_=wkflat[:, NW : 2 * NW]
                        )
                        nc.sync.dma_start(out=bias_sb[:], in_=bb[:])
                    r0 = r1

                # box-sums, emitted per DMA chunk so they overlap the loads:
                # xv[j]   = xp[j] + xp[j+58] + xp[j+116]   (rows 0..55)
                # xh[j]   = xp[j] + xp[j+1] + xp[j+2]      (rows 0..57, garbage
                #                                           at cols 56/57 unused)
                # xbox[j] = xv[j] + xv[j+1] + xv[j+2]      (rows 0..55, garbage
                #                                           at cols 56/57 unused)
                xvt = vpool.tile([CI, NV], BF16)
                xv = vpool.tile([CI, NV], BF16)
                xht = hpool.tile([CI, NPAD], BF16)
                xh = hpool.tile([CI, NPAD], BF16)
                xbt = bpool.tile([CI, NV], BF16)
                xb = bpool.tile([CI, NV], BF16)
                bounds = []
                for r1 in row_chunks:
                    bounds.extend(SUM_SPLITS.get(r1, [r1]) if b == 0 else [r1])
                v0 = h0r = 0
                for s1 in bounds:
                    v1 = H if s1 == HP else s1 - 2    # xv rows ready
                    h1 = s1                           # xh rows ready
                    a, z = v0 * WP, v1 * WP
                    nc.vector.tensor_add(
                        xvt[:, a:z], xp_sb[:, a:z], xp_sb[:, a + WP : z + WP]
                    )
                    nc.vector.tensor_add(
                        xv[:, a:z], xvt[:, a:z], xp_sb[:, a + 2 * WP : z + 2 * WP]
                    )
                    a, z = h0r * WP, h1 * WP - 2
                    nc.vector.tensor_add(
                        xht[:, a:z], xp_sb[:, a:z], xp_sb[:, a + 1 : z + 1]
                    )
                    nc.vector.tensor_add(
                        xh[:, a:z], xht[:, a:z], xp_sb[:, a + 2 : z + 2]
                    )
                    a, z = v0 * WP, v1 * WP - 2
                    nc.vector.tensor_add(
                        xbt[:, a:z], xv[:, a:z], xv[:, a + 1 : z + 1]
                    )
                    nc.vector.tensor_add(
                        xb[:, a:z], xbt[:, a:z], xv[:, a + 2 : z + 2]
                    )
                    v0, h0r = v1, h1

                xv3 = xv.rearrange("i (h w) -> i h w", w=WP)   # [128, 56, 58]
                xh3 = xh.rearrange("i (h w) -> i h w", w=WP)   # [128, 58, 58]
                xb3 = xb.rearrange("i (h w) -> i h w", w=WP)   # [128, 56, 58]

                youts = {}

                def mm5(ps, half, h0, nr, kl4=kl4, xv3=xv3, xh3=xh3, xb3=xb3):
                    # 5 taps in data-readiness order: xv, xv, xh, xh, xbox
                    nc.tensor.matmul(
                        ps[:], kl4[:, half, 0, :],
                        xv3[:, h0 : h0 + nr, 0:W], start=True, stop=False,
                    )
                    nc.tensor.matmul(
                        ps[:], kl4[:, half, 1, :],
                        xv3[:, h0 : h0 + nr, 2 : 2 + W], start=False, stop=False,
                    )
                    nc.tensor.matmul(
                        ps[:], kl4[:, half, 2, :],
                        xh3[:, h0 : h0 + nr, 0:W], start=False, stop=False,
                    )
                    nc.tensor.matmul(
                        ps[:], kl4[:, half, 3, :],
                        xh3[:, h0 + 2 : h0 + 2 + nr, 0:W], start=False, stop=False,
                    )
                    nc.tensor.matmul(
                        ps[:], kl4[:, half, 4, :],
                        xb3[:, h0 : h0 + nr, 0:W], start=False, stop=True,
                    )

                def emit(half, t, b=b, youts=youts):
                    if half not in youts:
                        youts[half] = ypool.tile(
                            [128, NPIX], BF16, name=f"yout_{b}_{half}", tag="yout"
                        )
                    yout = youts[half]
                    h0 = t * ROWT
                    ps = pspool.tile([128, NTILE], F32, name=f"ps_{b}_{half}_{t}", tag="ps")
                    mm5(ps, half, h0, ROWT)
                    last_block = b == BPC - 1 and half == 1
                    if last_block and t == NT - 1:
                        # final tile: split activation + store into halves so
                        # the second store's issue overlaps the first's and
                        # the kernel tail only waits on a 224-column DMA
                        c0 = t * NTILE
                        for p0, p1 in ((0, NTILE // 2), (NTILE // 2, NTILE)):
                            nc.scalar.activation(
                                yout[:, c0 + p0 : c0 + p1],
                                ps[:, p0:p1],
                                mybir.ActivationFunctionType.Identity,
                                bias=bias_sb[:, half : half + 1],
                            )
                            nc.sync.dma_start(
                                out=yflat[
                                    b, half * 128 : half * 128 + 128, c0 + p0 : c0 + p1
                                ],
                                in_=yout[:, c0 + p0 : c0 + p1],
                            )
                        return
                    nc.scalar.activation(
                        yout[:, t * NTILE : (t + 1) * NTILE],
                        ps[:],
                        mybir.ActivationFunctionType.Identity,
                        bias=bias_sb[:, half : half + 1],
                    )
                    if t == 3:
                        nc.sync.dma_start(
                            out=yflat[b, half * 128 : half * 128 + 128, 0 : 4 * NTILE],
                            in_=yout[:, 0 : 4 * NTILE],
                        )
                    elif t >= 4 and last_block:
                        # final block: per-tile stores so the kernel tail
                        # only waits on small DMAs
                        nc.sync.dma_start(
                            out=yflat[
                                b,
                                half * 128 : half * 128 + 128,
                                t * NTILE : (t + 1) * NTILE,
                            ],
                            in_=yout[:, t * NTILE : (t + 1) * NTILE],
                        )
                    if t == NT - 1 and not last_block:
                        nc.sync.dma_start(
                            out=yflat[b, half * 128 : half * 128 + 128, 4 * NTILE : NPIX],
                            in_=yout[:, 4 * NTILE : NPIX],
                        )

                if b == 0:
                    # image 0: interleave halves so each arriving row chunk
                    # immediately unlocks two tiles of PE work
                    order = [(h, t) for t in range(NT) for h in range(2)]
                elif b == BPC - 1:
                    # last image: stop half 1 before its final 8-row tile;
                    # rows 48-55 are emitted below as two 4-row tiles so the
                    # kernel tail waits on a 224-column activation + store
                    order = [(0, t) for t in range(NT)] + [(1, t) for t in range(NT - 1)]
                else:
                    order = [(h, t) for h in range(2) for t in range(NT)]
                for half, t in order:
                    emit(half, t)
                if b == BPC - 1:
                    yout1 = youts[1]
                    for k in range(2):
                        h0 = (NT - 1) * ROWT + 4 * k
                        c0 = h0 * W
                        ps = pspool.tile([128, 4 * W], F32, name=f"ps_tail_{k}", tag="ps")
                        mm5(ps, 1, h0, 4)
                        nc.scalar.activation(
                            yout1[:, c0 : c0 + 4 * W], ps[:],
                            mybir.ActivationFunctionType.Identity,
                            bias=bias_sb[:, 1:2],
                        )
                        nc.sync.dma_start(
                            out=yflat[b, 128:256, c0 : c0 + 4 * W],
                            in_=yout1[:, c0 : c0 + 4 * W],
                        )

            # read the warm PSUM bank at the very end so the warmup matmuls
            # are never dead-code-eliminated but gate nothing
            warm_out = cpool.tile([128, 32], F32)
            nc.scalar.activation(
                warm_out[:], warm_ps[:, 0:32], mybir.ActivationFunctionType.Copy
            )
    nc.finalize()
    return nc


_NC_CACHE = {}


def _get_nc():
    if "nc" not in _NC_CACHE:
        _NC_CACHE["nc"] = build_nc()
    return _NC_CACHE["nc"]


def make_in_maps(x, weight, bias):
    x = np.asarray(x, dtype=np.float32)
    weight = np.asarray(weight, dtype=np.float32)
    bias = np.asarray(bias, dtype=np.float32)

    krow = weight.sum(axis=3)  # [O, I, 3]
    kcol = weight.sum(axis=2)  # [O, I, 3]
    taps = [
        krow[:, :, 0] - krow[:, :, 1],   # d0 @ xv(w-1)
        krow[:, :, 2] - krow[:, :, 1],   # d2 @ xv(w+1)
        kcol[:, :, 0] - kcol[:, :, 1],   # e0 @ xh(h-1)
        kcol[:, :, 2] - kcol[:, :, 1],   # e2 @ xh(h+1)
        krow[:, :, 1] + kcol[:, :, 1],   # W1 @ xbox
    ]
    klhs = np.empty((CI, 2, NTAP, 128), np.float32)
    for half in range(2):
        o0 = half * 128
        for t, tap in enumerate(taps):
            klhs[:, half, t, :] = tap[o0 : o0 + 128, :].T
    klhs = klhs.astype(ml_dtypes.bfloat16)

    xp = np.zeros((B, CI, HP, WP), np.float32)
    xp[:, :, 1 : H + 1, 1 : W + 1] = x
    xp = xp.astype(ml_dtypes.bfloat16)

    bias2 = np.ascontiguousarray(bias.reshape(2, 128).T)  # [128, 2] f32

    return [
        {"xp": xp[c * BPC : (c + 1) * BPC], "klhs": klhs, "bias2": bias2}
        for c in range(NCORES)
    ]


def run(in_maps, **kwargs):
    nc = _get_nc()
    return run_bass_kernel_spmd(nc, in_maps, list(range(NCORES)), **kwargs)


def kernel(x, weight, bias):
    res = run(make_in_maps(x, weight, bias))
    return np.concatenate(
        [res.results[c]["y"].astype(np.float32) for c in range(NCORES)], axis=0
    )
